# revision 20
# baseline (speedup 1.0000x reference)
"""Causal self-attention (12 heads, T=1024, C=768, prefix P=4) on 8 TRN2 cores.

Sharding: data-parallel over batch B=8 -> one batch element per NeuronCore.
No collectives. Weights are replicated to every core.

Per-core kernel (all fp32):
  qkv projection split by destination layout:
    qT,kT  [128, T] per head-pair (transposed layout) = w_attn_slice.T @ xT
    v      [T, 12*65] natural layout (65th col per head = 1.0 for the
           softmax denominator), = xT_slice.T @ w_v
  prefix k/v (4 positions) are appended at the END of the kv axis, so the
  causal structure is block lower-triangular in (kv-chunk, t-chunk) space:
    scores^T tile (r, window c): psum = kT_slice.T @ qT_window  [128kv, 512t]
    e = exp(0.125 * psum)  (no max subtraction: |scores| ~ 2)
    diagonal band tiles multiplied by a 128x128 triangular 0/1 mask;
    fully-masked columns are never computed nor read.
  AV: y[tchunk] = sum_r e^T(r).T @ v_aug(r)  -> psum [128t, 65]
    col 64 = softmax denominator; normalize via DVE reciprocal +
    per-partition tensor_scalar_mul.  Two heads share a [128,128] y tile,
    one PE transpose each -> yT pair tiles [128, T].
  out = yT.T @ w_proj + b_proj  -> [T, 768] -> DMA out.
"""

import numpy as np
from contextlib import ExitStack

import concourse.bass as bass
import concourse.mybir as mybir
import concourse.tile as tile
from concourse import bacc
from concourse.bass_utils import run_bass_kernel_spmd

F32 = mybir.dt.float32
F32R = mybir.dt.float32r
F16 = mybir.dt.float16
N_CORES = 8
T, C, H, D, PFX = 1024, 768, 12, 64, 4
NPAIR = H // 2          # 6 head pairs
KC = C // 128           # 6 contraction chunks
W = 512                 # T window for scores
NW = T // W             # 2 windows
TCH = T // 128          # 8 T chunks
EXP = mybir.ActivationFunctionType.Exp
IDENT = mybir.ActivationFunctionType.Identity
SCALE = 1.0 / np.sqrt(D)


def _build():
    nc = bacc.Bacc("TRN2", target_bir_lowering=False, debug=False,
                   num_devices=N_CORES)
    xT_d = nc.declare_dram_parameter("xT", [C, T], F32, isOutput=False)
    wqk_d = nc.declare_dram_parameter("w_qk", [C, 2 * C], F16, isOutput=False)
    wv_d = nc.declare_dram_parameter("w_v", [C, C], F16, isOutput=False)
    wp_d = nc.declare_dram_parameter("w_p", [C, C], F32, isOutput=False)
    bqk_d = nc.declare_dram_parameter("b_qk", [128, 12], F32, isOutput=False)
    bv_d = nc.declare_dram_parameter("bv_bc", [128, C], F32, isOutput=False)
    bp_d = nc.declare_dram_parameter("bp_bc", [128, C], F32, isOutput=False)
    kTc_d = nc.declare_dram_parameter("kTc", [C, PFX], F16, isOutput=False)
    vc_d = nc.declare_dram_parameter("vc_aug", [PFX, H, 128], F16, isOutput=False)
    tri_d = nc.declare_dram_parameter("tri", [128, 128], F16, isOutput=False)
    ones_d = nc.declare_dram_parameter("ones2", [66, 64], F32, isOutput=False)
    zer_d = nc.declare_dram_parameter("zeros", [128, T + PFX], F16, isOutput=False)
    out_d = nc.declare_dram_parameter("out", [T, C], F32, isOutput=True)

    with tile.TileContext(nc) as tc, ExitStack() as ctx:
        pers = ctx.enter_context(tc.tile_pool(name="pers", bufs=1))
        wqkp = ctx.enter_context(tc.tile_pool(name="wqkp", bufs=12))
        qkp = ctx.enter_context(tc.tile_pool(name="qkp", bufs=2))
        ep = ctx.enter_context(tc.tile_pool(name="ep", bufs=26))
        khp = ctx.enter_context(tc.tile_pool(name="khp", bufs=4))
        epp = ctx.enter_context(tc.tile_pool(name="epp", bufs=6))
        sbp = ctx.enter_context(tc.tile_pool(name="sbp", bufs=4))
        drp = ctx.enter_context(tc.tile_pool(name="drp", bufs=4))
        dram = ctx.enter_context(tc.tile_pool(name="dram", bufs=4, space="DRAM"))
        rwp = ctx.enter_context(tc.tile_pool(name="rwp", bufs=4))
        op = ctx.enter_context(tc.tile_pool(name="op", bufs=2))
        ps = ctx.enter_context(tc.tile_pool(name="ps", bufs=3, space="PSUM"))
        psp = ctx.enter_context(tc.tile_pool(name="psp", bufs=1, space="PSUM"))
        pyp = ctx.enter_context(tc.tile_pool(name="pyp", bufs=3, space="PSUM"))
        pbp = ctx.enter_context(tc.tile_pool(name="pbp", bufs=1, space="PSUM"))

        # ---- persistent loads (x and w_v first so v-projection starts ASAP) --
        xt = []
        for k in range(KC):
            t_ = pers.tile([128, T], F32, tag=f"xt{k}")
            nc.sync.dma_start(t_[:, 0:512], xT_d[128 * k:128 * k + 128, 0:512])
            nc.sync.dma_start(t_[:, 512:T], xT_d[128 * k:128 * k + 128, 512:T])
            xt.append(t_)
        wv = []
        for k in range(KC):
            t_ = pers.tile([128, C], F16, tag=f"wv{k}")
            nc.sync.dma_start(t_[:], wv_d[128 * k:128 * k + 128, :])
            wv.append(t_)
        xtb = []
        for k in range(KC):
            t_ = pers.tile([128, T], F16, tag=f"xtb{k}")
            nc.vector.tensor_copy(t_[:], xt[k][:])
            xtb.append(t_)
        bqk = pers.tile([128, 12], F32, tag="bqk")
        nc.sync.dma_start(bqk[:], bqk_d[:])
        bv = pers.tile([128, C], F32, tag="bv")
        nc.sync.dma_start(bv[:], bv_d[:])
        tri = pers.tile([128, 128], F16, tag="tri")
        nc.sync.dma_start(tri[:], tri_d[:])
        vc = pers.tile([PFX, H, 128], F16, tag="vc")
        nc.sync.dma_start(vc[:], vc_d[:])
        ones2 = pers.tile([66, 64], F32R, tag="ones2")
        nc.sync.dma_start(ones2[:], ones_d[:].bitcast(F32R))
        wp = []
        for k in range(KC):
            t_ = pers.tile([128, C], F32R, tag=f"wp{k}")
            nc.sync.dma_start(t_[:], wp_d[128 * k:128 * k + 128, :].bitcast(F32R))
            wp.append(t_)
        bp = pers.tile([128, C], F32, tag="bp")
        nc.sync.dma_start(bp[:], bp_d[:])

        yT = [pers.tile([128, T], F32R, tag=f"yT{p}", name=f"yT{p}")
              for p in range(NPAIR)]

        # ---- v projection: [128, 12, 65] fp16, 65th col per head = 1.0 ----
        vt = []
        for mt in range(TCH):
            v_ = pers.tile([128, H, 128], F16, tag=f"v{mt}")
            nc.vector.memset(v_[:, :, 64:65], 1.0)
            nc.sync.dma_start(
                v_[:, :, 65:128],
                zer_d[:, 0:H * 63].rearrange("a (h c) -> a h c", h=H))
            for n0, nsz in ((0, 512), (512, 256)):
                pv = ps.tile([128, 512], F32, tag="ps", name=f"pv{mt}_{n0}")
                for k in range(KC):
                    nc.tensor.matmul(pv[:, :nsz],
                                     xtb[k][:, 128 * mt:128 * mt + 128],
                                     wv[k][:, n0:n0 + nsz],
                                     start=(k == 0), stop=(k == KC - 1))
                h0, hn = n0 // 64, nsz // 64
                nc.vector.tensor_add(
                    v_[:, h0:h0 + hn, 0:64],
                    pv[:, :nsz].rearrange("a (h d) -> a h d", d=64),
                    bv[:, n0:n0 + nsz].rearrange("a (h d) -> a h d", d=64))
            vt.append(v_)

        # ---- phases ----
        qk_tiles = {}
        ets = {}
        etps = {}
        pys = {}

        def qkproj(p):
            wq = []
            for k in range(KC):
                t_ = wqkp.tile([128, 2, 128], F16, tag="wqk", name=f"wq{p}_{k}")
                src = wqk_d[128 * k:128 * k + 128, :].rearrange(
                    "a (s b) -> a s b", s=2)[:, :, 128 * p:128 * p + 128]
                nc.sync.dma_start(t_[:], src)
                wq.append(t_)
            qT = qkp.tile([128, T], F16, tag="qT", name=f"qT{p}")
            kh = [khp.tile([128, T + PFX], F16, tag="kh", name=f"kh{p}_{s}")
                  for s in range(2)]
            qk_tiles[p] = (qT, kh)
            for s in range(2):
                # head s's k-features live at rows 64s..64s+63 (aligned with
                # its q rows in the pair tile); other 64 rows are zero.
                nc.sync.dma_start(kh[s][64 - 64 * s:128 - 64 * s, :],
                                  zer_d[0:64, :])
                nc.sync.dma_start(
                    kh[s][64 * s:64 * s + 64, T:T + PFX],
                    kTc_d[128 * p + 64 * s:128 * p + 64 * s + 64, :])
            for half in range(2):
                for w in range(NW):
                    pq = ps.tile([128, 512], F32, tag="ps",
                                 name=f"pq{p}_{half}_{w}")
                    for k in range(KC):
                        nc.tensor.matmul(pq[:], wq[k][:, half, :],
                                         xtb[k][:, W * w:W * w + W],
                                         start=(k == 0), stop=(k == KC - 1))
                    if half == 0:
                        nc.vector.tensor_scalar_add(
                            qT[:, W * w:W * w + W], pq[:],
                            bqk[:, p:p + 1])
                    else:
                        for s in range(2):
                            nc.vector.tensor_scalar_add(
                                kh[s][64 * s:64 * s + 64, W * w:W * w + W],
                                pq[64 * s:64 * s + 64, :],
                                bqk[64 * s:64 * s + 64, 6 + p:7 + p])

        def scores(p, s):
            qT, kh = qk_tiles[p]
            qs = qT
            ks = kh[s]
            for c in range(NW):
                for r in range(4 * c + 4):
                    e_ = ep.tile([128, W], F16, tag="et", name=f"et{p}_{s}_{c}_{r}")
                    pss = ps.tile([128, 512], F32, tag="ps",
                                  name=f"pss{p}_{s}_{c}_{r}")
                    if r >= 4 * c:  # diagonal band tile
                        j0 = 128 * r - W * c
                        nc.tensor.matmul(pss[:, j0:W], ks[:, 128 * r:128 * r + 128],
                                         qs[:, W * c + j0:W * (c + 1)],
                                         start=True, stop=True)
                        nc.scalar.activation(e_[:, j0:W], pss[:, j0:W], EXP,
                                             scale=float(SCALE))
                        nc.vector.tensor_mul(e_[:, j0:j0 + 128],
                                             e_[:, j0:j0 + 128], tri[:])
                    else:
                        nc.tensor.matmul(pss[:], ks[:, 128 * r:128 * r + 128],
                                         qs[:, W * c:W * (c + 1)],
                                         start=True, stop=True)
                        nc.scalar.activation(e_[:], pss[:], EXP,
                                             scale=float(SCALE))
                    ets[(p, s, c, r)] = e_
                pp = psp.tile([PFX, 512], F32, tag="psp", name=f"pp{p}_{s}_{c}")
                nc.tensor.matmul(pp[:], ks[:, T:T + PFX],
                                 qs[:, W * c:W * (c + 1)], start=True, stop=True)
                ep_ = epp.tile([PFX, W], F16, tag="etp", name=f"etp{p}_{s}_{c}")
                nc.scalar.activation(ep_[:], pp[:], EXP, scale=float(SCALE))
                etps[(p, s, c)] = ep_

        def av(p, s, c):
            """y^T accumulation: py[0:65, t] = [y(64 dims); denom] for head."""
            h = 2 * p + s
            py = pyp.tile([128, W], F32, tag="py", name=f"py{p}_{s}_{c}")
            pys[(p, s, c)] = py
            dst = py[:, :]
            for r in range(4 * c + 4):
                tstart = max(0, 128 * r - W * c)
                nc.tensor.matmul(dst[:, tstart:W],
                                 vt[r][:, h, :],
                                 ets[(p, s, c, r)][:, tstart:W],
                                 start=(r == 0), stop=False)
            nc.tensor.matmul(dst[:, :], vc[:, h, :],
                             etps[(p, s, c)][:, :], start=False, stop=True)

        def norm(p, c):
            """Normalize both heads of the pair for window c into yT[p].

            The denom row [1, W] is reshaped to [128, 4] via DMA so the
            reciprocal runs partition-parallel, DMA'd back as an f32r row,
            broadcast to [64, W] via a K=2 f32r matmul, and multiplied in
            with a shifted output base for head 1."""
            for s in range(2):
                py = pys[(p, s, c)]
                sb = sbp.tile([66, W], F32, tag="sb", name=f"sb{p}_{s}_{c}")
                nc.vector.tensor_copy(sb[:], py[0:66, :])
                dsc = dram.tile([2, W], F32, tag="dsc", name=f"dsc{p}_{s}_{c}")
                nc.sync.dma_start(dsc[0:1, :], sb[64:65, :])
                drow = drp.tile([128, 8], F32, tag="dr", name=f"dr{p}_{s}_{c}")
                nc.vector.memset(drow[:, 4:8], 1.0)
                nc.sync.dma_start(
                    drow[:, 0:4],
                    dsc[0:1, :].rearrange("a (p f) -> (a p) f", p=128))
                rrec = drp.tile([128, 8], F32, tag="rr", name=f"rr{p}_{s}_{c}")
                nc.vector.reciprocal(rrec[:], drow[:])
                dsc2 = dram.tile([2, W], F32, tag="dsc2", name=f"ds2{p}_{s}_{c}")
                nc.sync.dma_start(
                    dsc2[:].rearrange("r (p f) -> p r f", p=128),
                    rrec[:].rearrange("p (r f) -> p r f", r=2))
                rrow = rwp.tile([66, W], F32R, tag="rw", name=f"rw{p}_{s}_{c}")
                nc.sync.dma_start(rrow[64:66, :], dsc2[:].bitcast(F32R))
                pb = pbp.tile([64, W], F32, tag="pb", name=f"pd{p}_{s}_{c}")
                nc.tensor.matmul(pb[:], ones2[64:66, :], rrow[64:66, :],
                                 start=True, stop=True)
                nc.vector.tensor_mul(yT[p][64 * s:64 * s + 64, W * c:W * c + W],
                                     sb[0:64, :], pb[:])

        def outproj(mts):
            for mt in mts:
                osb = op.tile([128, C], F32, tag="osb", name=f"osb{mt}")
                for n0, nsz in ((0, 512), (512, 256)):
                    po = ps.tile([128, 512], F32, tag="ps", name=f"po{mt}_{n0}")
                    for kp in range(NPAIR):
                        nc.tensor.matmul(po[:, :nsz],
                                         yT[kp][:, 128 * mt:128 * mt + 128],
                                         wp[kp][:, n0:n0 + nsz],
                                         start=(kp == 0), stop=(kp == NPAIR - 1))
                    nc.vector.tensor_add(osb[:, n0:n0 + nsz], po[:, :nsz],
                                         bp[:, n0:n0 + nsz])
                nc.sync.dma_start(out_d[128 * mt:128 * mt + 128, :], osb[:])

        # ---- emission schedule ----
        qkproj(0)
        scores(0, 0)
        for p in range(NPAIR):
            scores(p, 1)
            if p < NPAIR - 1:
                qkproj(p + 1)
            for c in range(NW):
                av(p, 0, c)
                av(p, 1, c)
                norm(p, c)
            if p < NPAIR - 1:
                scores(p + 1, 0)
        outproj(range(TCH))

    nc.finalize()
    return nc


def _ones2():
    o = np.zeros((66, 64), np.float32)
    o[64, :] = 1.0
    return o


def _prep_inputs(x, kv_cvec, w_attn, b_attn, w_proj, b_proj):
    x = np.asarray(x, np.float32)
    kv_cvec = np.asarray(kv_cvec, np.float32)
    w_attn = np.asarray(w_attn, np.float32)
    b_attn = np.asarray(b_attn, np.float32)
    w_proj = np.asarray(w_proj, np.float32)
    b_proj = np.asarray(b_proj, np.float32)

    shared = {
        "w_qk": np.ascontiguousarray(w_attn[:, :2 * C]).astype(np.float16),
        "w_v": np.ascontiguousarray(w_attn[:, 2 * C:]).astype(np.float16),
        "w_p": np.ascontiguousarray(w_proj),
        "b_qk": np.ascontiguousarray(b_attn[:2 * C].reshape(12, 128).T),
        "bv_bc": np.ascontiguousarray(
            np.broadcast_to(b_attn[2 * C:], (128, C))),
        "bp_bc": np.ascontiguousarray(np.broadcast_to(b_proj, (128, C))),
        "tri": (np.arange(128)[:, None] <= np.arange(128)[None, :]
                ).astype(np.float16),
        "ones2": _ones2(),
        "zeros": np.zeros((128, T + PFX), np.float16),
    }
    in_maps = []
    for b in range(N_CORES):
        vc_aug = np.zeros((PFX, H, 128), np.float32)
        vc_aug[:, :, :64] = kv_cvec[b][:, C:].reshape(PFX, H, D)
        vc_aug[:, :, 64] = 1.0
        m = dict(shared)
        m["xT"] = np.ascontiguousarray(x[b].T)
        m["kTc"] = np.ascontiguousarray(kv_cvec[b][:, :C].T
                                        ).astype(np.float16)
        m["vc_aug"] = vc_aug.astype(np.float16)
        in_maps.append(m)
    return in_maps


_NC_CACHE = {}


def run_hw(trace=False, **inputs):
    """Build+compile+run on 8 NeuronCores; returns (out [8,1024,768], results)."""
    if "nc" not in _NC_CACHE:
        _NC_CACHE["nc"] = _build()
    nc = _NC_CACHE["nc"]
    in_maps = _prep_inputs(**inputs)
    res = run_bass_kernel_spmd(nc, in_maps, list(range(N_CORES)), trace=trace)
    out = np.stack([res.results[b]["out"] for b in range(N_CORES)])
    return out, res


def kernel(**inputs):
    out, _ = run_hw(trace=False, **inputs)
    return out


# revision 21
# speedup vs baseline: 1.0589x; 1.0589x over previous
"""Causal self-attention (12 heads, T=1024, C=768, prefix P=4) on 8 TRN2 cores.

Sharding: data-parallel over batch B=8 -> one batch element per NeuronCore.
No collectives. Weights are replicated to every core.

Per-core kernel (all fp32):
  qkv projection split by destination layout:
    qT,kT  [128, T] per head-pair (transposed layout) = w_attn_slice.T @ xT
    v      [T, 12*65] natural layout (65th col per head = 1.0 for the
           softmax denominator), = xT_slice.T @ w_v
  prefix k/v (4 positions) are appended at the END of the kv axis, so the
  causal structure is block lower-triangular in (kv-chunk, t-chunk) space:
    scores^T tile (r, window c): psum = kT_slice.T @ qT_window  [128kv, 512t]
    e = exp(0.125 * psum)  (no max subtraction: |scores| ~ 2)
    diagonal band tiles multiplied by a 128x128 triangular 0/1 mask;
    fully-masked columns are never computed nor read.
  AV: y[tchunk] = sum_r e^T(r).T @ v_aug(r)  -> psum [128t, 65]
    col 64 = softmax denominator; normalize via DVE reciprocal +
    per-partition tensor_scalar_mul.  Two heads share a [128,128] y tile,
    one PE transpose each -> yT pair tiles [128, T].
  out = yT.T @ w_proj + b_proj  -> [T, 768] -> DMA out.
"""

import numpy as np
from contextlib import ExitStack

import concourse.bass as bass
import concourse.mybir as mybir
import concourse.tile as tile
from concourse import bacc
from concourse.bass_utils import run_bass_kernel_spmd

F32 = mybir.dt.float32
F32R = mybir.dt.float32r
F16 = mybir.dt.float16
N_CORES = 8
T, C, H, D, PFX = 1024, 768, 12, 64, 4
NPAIR = H // 2          # 6 head pairs
KC = C // 128           # 6 contraction chunks
W = 512                 # T window for scores
NW = T // W             # 2 windows
TCH = T // 128          # 8 T chunks
EXP = mybir.ActivationFunctionType.Exp
IDENT = mybir.ActivationFunctionType.Identity
SCALE = 1.0 / np.sqrt(D)


def _build():
    nc = bacc.Bacc("TRN2", target_bir_lowering=False, debug=False,
                   num_devices=N_CORES)
    xT_d = nc.declare_dram_parameter("xT", [C, T], F32, isOutput=False)
    wqk_d = nc.declare_dram_parameter("w_qk", [C, 2 * C], F16, isOutput=False)
    wv_d = nc.declare_dram_parameter("w_v", [C, C], F16, isOutput=False)
    wp_d = nc.declare_dram_parameter("w_p", [C, C], F32, isOutput=False)
    bqk_d = nc.declare_dram_parameter("b_qk", [128, 12], F32, isOutput=False)
    bv_d = nc.declare_dram_parameter("bv_bc", [128, C], F32, isOutput=False)
    bp_d = nc.declare_dram_parameter("bp_bc", [128, C], F32, isOutput=False)
    kTc_d = nc.declare_dram_parameter("kTc", [C, PFX], F16, isOutput=False)
    vc_d = nc.declare_dram_parameter("vc_aug", [PFX, H, 128], F16, isOutput=False)
    tri_d = nc.declare_dram_parameter("tri", [128, 128], F16, isOutput=False)
    ones_d = nc.declare_dram_parameter("ones2", [66, 64], F32, isOutput=False)
    zer_d = nc.declare_dram_parameter("zeros", [128, T + PFX], F16, isOutput=False)
    out_d = nc.declare_dram_parameter("out", [T, C], F32, isOutput=True)

    with tile.TileContext(nc) as tc, ExitStack() as ctx:
        pers = ctx.enter_context(tc.tile_pool(name="pers", bufs=1))
        wqkp = ctx.enter_context(tc.tile_pool(name="wqkp", bufs=12))
        qkp = ctx.enter_context(tc.tile_pool(name="qkp", bufs=2))
        ep = ctx.enter_context(tc.tile_pool(name="ep", bufs=26))
        khp = ctx.enter_context(tc.tile_pool(name="khp", bufs=4))
        epp = ctx.enter_context(tc.tile_pool(name="epp", bufs=6))
        sbp = ctx.enter_context(tc.tile_pool(name="sbp", bufs=4))
        drp = ctx.enter_context(tc.tile_pool(name="drp", bufs=4))
        dram = ctx.enter_context(tc.tile_pool(name="dram", bufs=4, space="DRAM"))
        rwp = ctx.enter_context(tc.tile_pool(name="rwp", bufs=4))
        op = ctx.enter_context(tc.tile_pool(name="op", bufs=2))
        ps = ctx.enter_context(tc.tile_pool(name="ps", bufs=3, space="PSUM"))
        psp = ctx.enter_context(tc.tile_pool(name="psp", bufs=1, space="PSUM"))
        pyp = ctx.enter_context(tc.tile_pool(name="pyp", bufs=3, space="PSUM"))
        pbp = ctx.enter_context(tc.tile_pool(name="pbp", bufs=1, space="PSUM"))

        # ---- persistent loads (x and w_v first so v-projection starts ASAP) --
        xt = []
        for k in range(KC):
            t_ = pers.tile([128, T], F32, tag=f"xt{k}")
            nc.sync.dma_start(t_[:, 0:512], xT_d[128 * k:128 * k + 128, 0:512])
            nc.sync.dma_start(t_[:, 512:T], xT_d[128 * k:128 * k + 128, 512:T])
            xt.append(t_)
        wv = []
        for k in range(KC):
            t_ = pers.tile([128, C], F16, tag=f"wv{k}")
            nc.sync.dma_start(t_[:], wv_d[128 * k:128 * k + 128, :])
            wv.append(t_)
        xtb = []
        for k in range(KC):
            t_ = pers.tile([128, T], F16, tag=f"xtb{k}")
            nc.vector.tensor_copy(t_[:], xt[k][:])
            xtb.append(t_)
        bqk = pers.tile([128, 12], F32, tag="bqk")
        nc.sync.dma_start(bqk[:], bqk_d[:])
        bv = pers.tile([128, C], F32, tag="bv")
        nc.sync.dma_start(bv[:], bv_d[:])
        tri = pers.tile([128, 128], F16, tag="tri")
        nc.sync.dma_start(tri[:], tri_d[:])
        vc = pers.tile([PFX, H, 128], F16, tag="vc")
        nc.sync.dma_start(vc[:], vc_d[:])
        ones2 = pers.tile([66, 64], F32R, tag="ones2")
        nc.sync.dma_start(ones2[:], ones_d[:].bitcast(F32R))
        wp = []
        for k in range(KC):
            t_ = pers.tile([128, C], F32R, tag=f"wp{k}")
            nc.sync.dma_start(t_[:], wp_d[128 * k:128 * k + 128, :].bitcast(F32R))
            wp.append(t_)
        bp = pers.tile([128, C], F32, tag="bp")
        nc.sync.dma_start(bp[:], bp_d[:])

        yT = [pers.tile([128, T], F32R, tag=f"yT{p}", name=f"yT{p}")
              for p in range(NPAIR)]

        # ---- v projection: [128, 12, 65] fp16, 65th col per head = 1.0 ----
        vt = []
        for mt in range(TCH):
            v_ = pers.tile([128, H, 128], F16, tag=f"v{mt}")
            nc.vector.memset(v_[:, :, 64:65], 1.0)
            nc.vector.memset(v_[:, :, 65:128], 0.0)
            for n0, nsz in ((0, 512), (512, 256)):
                pv = ps.tile([128, 512], F32, tag="ps", name=f"pv{mt}_{n0}")
                for k in range(KC):
                    nc.tensor.matmul(pv[:, :nsz],
                                     xtb[k][:, 128 * mt:128 * mt + 128],
                                     wv[k][:, n0:n0 + nsz],
                                     start=(k == 0), stop=(k == KC - 1))
                h0, hn = n0 // 64, nsz // 64
                nc.vector.tensor_add(
                    v_[:, h0:h0 + hn, 0:64],
                    pv[:, :nsz].rearrange("a (h d) -> a h d", d=64),
                    bv[:, n0:n0 + nsz].rearrange("a (h d) -> a h d", d=64))
            vt.append(v_)

        # ---- phases ----
        qk_tiles = {}
        ets = {}
        etps = {}
        pys = {}

        def qkproj(p):
            wq = []
            for k in range(KC):
                t_ = wqkp.tile([128, 2, 128], F16, tag="wqk", name=f"wq{p}_{k}")
                src = wqk_d[128 * k:128 * k + 128, :].rearrange(
                    "a (s b) -> a s b", s=2)[:, :, 128 * p:128 * p + 128]
                nc.sync.dma_start(t_[:], src)
                wq.append(t_)
            qT = qkp.tile([128, T], F16, tag="qT", name=f"qT{p}")
            kh = [khp.tile([128, T + PFX], F16, tag="kh", name=f"kh{p}_{s}")
                  for s in range(2)]
            qk_tiles[p] = (qT, kh)
            for s in range(2):
                # head s's k-features live at rows 64s..64s+63 (aligned with
                # its q rows in the pair tile); other 64 rows are zero.
                nc.vector.memset(kh[s][64 - 64 * s:128 - 64 * s, :], 0.0)
                nc.sync.dma_start(
                    kh[s][64 * s:64 * s + 64, T:T + PFX],
                    kTc_d[128 * p + 64 * s:128 * p + 64 * s + 64, :])
            for half in range(2):
                for w in range(NW):
                    pq = ps.tile([128, 512], F32, tag="ps",
                                 name=f"pq{p}_{half}_{w}")
                    for k in range(KC):
                        nc.tensor.matmul(pq[:], wq[k][:, half, :],
                                         xtb[k][:, W * w:W * w + W],
                                         start=(k == 0), stop=(k == KC - 1))
                    if half == 0:
                        nc.vector.tensor_scalar_add(
                            qT[:, W * w:W * w + W], pq[:],
                            bqk[:, p:p + 1])
                    else:
                        for s in range(2):
                            nc.vector.tensor_scalar_add(
                                kh[s][64 * s:64 * s + 64, W * w:W * w + W],
                                pq[64 * s:64 * s + 64, :],
                                bqk[64 * s:64 * s + 64, 6 + p:7 + p])

        def scores(p, s):
            qT, kh = qk_tiles[p]
            qs = qT
            ks = kh[s]
            for c in range(NW):
                for r in range(4 * c + 4):
                    e_ = ep.tile([128, W], F16, tag="et", name=f"et{p}_{s}_{c}_{r}")
                    pss = ps.tile([128, 512], F32, tag="ps",
                                  name=f"pss{p}_{s}_{c}_{r}")
                    if r >= 4 * c:  # diagonal band tile
                        j0 = 128 * r - W * c
                        nc.tensor.matmul(pss[:, j0:W], ks[:, 128 * r:128 * r + 128],
                                         qs[:, W * c + j0:W * (c + 1)],
                                         start=True, stop=True)
                        nc.scalar.activation(e_[:, j0:W], pss[:, j0:W], EXP,
                                             scale=float(SCALE))
                        nc.vector.tensor_mul(e_[:, j0:j0 + 128],
                                             e_[:, j0:j0 + 128], tri[:])
                    else:
                        nc.tensor.matmul(pss[:], ks[:, 128 * r:128 * r + 128],
                                         qs[:, W * c:W * (c + 1)],
                                         start=True, stop=True)
                        nc.scalar.activation(e_[:], pss[:], EXP,
                                             scale=float(SCALE))
                    ets[(p, s, c, r)] = e_
                pp = psp.tile([PFX, 512], F32, tag="psp", name=f"pp{p}_{s}_{c}")
                nc.tensor.matmul(pp[:], ks[:, T:T + PFX],
                                 qs[:, W * c:W * (c + 1)], start=True, stop=True)
                ep_ = epp.tile([PFX, W], F16, tag="etp", name=f"etp{p}_{s}_{c}")
                nc.scalar.activation(ep_[:], pp[:], EXP, scale=float(SCALE))
                etps[(p, s, c)] = ep_

        def av(p, s, c):
            """y^T accumulation: py[0:65, t] = [y(64 dims); denom] for head."""
            h = 2 * p + s
            py = pyp.tile([128, W], F32, tag="py", name=f"py{p}_{s}_{c}")
            pys[(p, s, c)] = py
            dst = py[:, :]
            for r in range(4 * c + 4):
                tstart = max(0, 128 * r - W * c)
                nc.tensor.matmul(dst[:, tstart:W],
                                 vt[r][:, h, :],
                                 ets[(p, s, c, r)][:, tstart:W],
                                 start=(r == 0), stop=False)
            nc.tensor.matmul(dst[:, :], vc[:, h, :],
                             etps[(p, s, c)][:, :], start=False, stop=True)

        def norm(p, c):
            """Normalize both heads of the pair for window c into yT[p].

            The denom row [1, W] is reshaped to [128, 4] via DMA so the
            reciprocal runs partition-parallel, DMA'd back as an f32r row,
            broadcast to [64, W] via a K=2 f32r matmul, and multiplied in
            with a shifted output base for head 1."""
            for s in range(2):
                py = pys[(p, s, c)]
                sb = sbp.tile([66, W], F32, tag="sb", name=f"sb{p}_{s}_{c}")
                nc.vector.tensor_copy(sb[:], py[0:66, :])
                dsc = dram.tile([2, W], F32, tag="dsc", name=f"dsc{p}_{s}_{c}")
                nc.sync.dma_start(dsc[0:1, :], sb[64:65, :])
                drow = drp.tile([128, 8], F32, tag="dr", name=f"dr{p}_{s}_{c}")
                nc.vector.memset(drow[:, 4:8], 1.0)
                nc.sync.dma_start(
                    drow[:, 0:4],
                    dsc[0:1, :].rearrange("a (p f) -> (a p) f", p=128))
                rrec = drp.tile([128, 8], F32, tag="rr", name=f"rr{p}_{s}_{c}")
                nc.vector.reciprocal(rrec[:], drow[:])
                dsc2 = dram.tile([2, W], F32, tag="dsc2", name=f"ds2{p}_{s}_{c}")
                nc.sync.dma_start(
                    dsc2[:].rearrange("r (p f) -> p r f", p=128),
                    rrec[:].rearrange("p (r f) -> p r f", r=2))
                rrow = rwp.tile([66, W], F32R, tag="rw", name=f"rw{p}_{s}_{c}")
                nc.sync.dma_start(rrow[64:66, :], dsc2[:].bitcast(F32R))
                pb = pbp.tile([64, W], F32, tag="pb", name=f"pd{p}_{s}_{c}")
                nc.tensor.matmul(pb[:], ones2[64:66, :], rrow[64:66, :],
                                 start=True, stop=True)
                nc.vector.tensor_mul(yT[p][64 * s:64 * s + 64, W * c:W * c + W],
                                     sb[0:64, :], pb[:])

        def outproj(mts):
            for mt in mts:
                osb = op.tile([128, C], F32, tag="osb", name=f"osb{mt}")
                for n0, nsz in ((0, 512), (512, 256)):
                    po = ps.tile([128, 512], F32, tag="ps", name=f"po{mt}_{n0}")
                    for kp in range(NPAIR):
                        nc.tensor.matmul(po[:, :nsz],
                                         yT[kp][:, 128 * mt:128 * mt + 128],
                                         wp[kp][:, n0:n0 + nsz],
                                         start=(kp == 0), stop=(kp == NPAIR - 1))
                    nc.vector.tensor_add(osb[:, n0:n0 + nsz], po[:, :nsz],
                                         bp[:, n0:n0 + nsz])
                nc.sync.dma_start(out_d[128 * mt:128 * mt + 128, :], osb[:])

        # ---- emission schedule ----
        qkproj(0)
        scores(0, 0)
        for p in range(NPAIR):
            scores(p, 1)
            if p < NPAIR - 1:
                qkproj(p + 1)
            for c in range(NW):
                av(p, 0, c)
                av(p, 1, c)
                norm(p, c)
            if p < NPAIR - 1:
                scores(p + 1, 0)
        outproj(range(TCH))

    nc.finalize()
    return nc


def _ones2():
    o = np.zeros((66, 64), np.float32)
    o[64, :] = 1.0
    return o


def _prep_inputs(x, kv_cvec, w_attn, b_attn, w_proj, b_proj):
    x = np.asarray(x, np.float32)
    kv_cvec = np.asarray(kv_cvec, np.float32)
    w_attn = np.asarray(w_attn, np.float32)
    b_attn = np.asarray(b_attn, np.float32)
    w_proj = np.asarray(w_proj, np.float32)
    b_proj = np.asarray(b_proj, np.float32)

    shared = {
        "w_qk": np.ascontiguousarray(w_attn[:, :2 * C]).astype(np.float16),
        "w_v": np.ascontiguousarray(w_attn[:, 2 * C:]).astype(np.float16),
        "w_p": np.ascontiguousarray(w_proj),
        "b_qk": np.ascontiguousarray(b_attn[:2 * C].reshape(12, 128).T),
        "bv_bc": np.ascontiguousarray(
            np.broadcast_to(b_attn[2 * C:], (128, C))),
        "bp_bc": np.ascontiguousarray(np.broadcast_to(b_proj, (128, C))),
        "tri": (np.arange(128)[:, None] <= np.arange(128)[None, :]
                ).astype(np.float16),
        "ones2": _ones2(),
        "zeros": np.zeros((128, T + PFX), np.float16),
    }
    in_maps = []
    for b in range(N_CORES):
        vc_aug = np.zeros((PFX, H, 128), np.float32)
        vc_aug[:, :, :64] = kv_cvec[b][:, C:].reshape(PFX, H, D)
        vc_aug[:, :, 64] = 1.0
        m = dict(shared)
        m["xT"] = np.ascontiguousarray(x[b].T)
        m["kTc"] = np.ascontiguousarray(kv_cvec[b][:, :C].T
                                        ).astype(np.float16)
        m["vc_aug"] = vc_aug.astype(np.float16)
        in_maps.append(m)
    return in_maps


_NC_CACHE = {}


def run_hw(trace=False, **inputs):
    """Build+compile+run on 8 NeuronCores; returns (out [8,1024,768], results)."""
    if "nc" not in _NC_CACHE:
        _NC_CACHE["nc"] = _build()
    nc = _NC_CACHE["nc"]
    in_maps = _prep_inputs(**inputs)
    res = run_bass_kernel_spmd(nc, in_maps, list(range(N_CORES)), trace=trace)
    out = np.stack([res.results[b]["out"] for b in range(N_CORES)])
    return out, res


def kernel(**inputs):
    out, _ = run_hw(trace=False, **inputs)
    return out


# revision 22
# speedup vs baseline: 1.0766x; 1.0166x over previous
"""Causal self-attention (12 heads, T=1024, C=768, prefix P=4) on 8 TRN2 cores.

Sharding: data-parallel over batch B=8 -> one batch element per NeuronCore.
No collectives. Weights are replicated to every core.

Per-core kernel (all fp32):
  qkv projection split by destination layout:
    qT,kT  [128, T] per head-pair (transposed layout) = w_attn_slice.T @ xT
    v      [T, 12*65] natural layout (65th col per head = 1.0 for the
           softmax denominator), = xT_slice.T @ w_v
  prefix k/v (4 positions) are appended at the END of the kv axis, so the
  causal structure is block lower-triangular in (kv-chunk, t-chunk) space:
    scores^T tile (r, window c): psum = kT_slice.T @ qT_window  [128kv, 512t]
    e = exp(0.125 * psum)  (no max subtraction: |scores| ~ 2)
    diagonal band tiles multiplied by a 128x128 triangular 0/1 mask;
    fully-masked columns are never computed nor read.
  AV: y[tchunk] = sum_r e^T(r).T @ v_aug(r)  -> psum [128t, 65]
    col 64 = softmax denominator; normalize via DVE reciprocal +
    per-partition tensor_scalar_mul.  Two heads share a [128,128] y tile,
    one PE transpose each -> yT pair tiles [128, T].
  out = yT.T @ w_proj + b_proj  -> [T, 768] -> DMA out.
"""

import numpy as np
from contextlib import ExitStack

import concourse.bass as bass
import concourse.mybir as mybir
import concourse.tile as tile
from concourse import bacc
from concourse.bass_utils import run_bass_kernel_spmd

F32 = mybir.dt.float32
F32R = mybir.dt.float32r
F16 = mybir.dt.float16
N_CORES = 8
T, C, H, D, PFX = 1024, 768, 12, 64, 4
NPAIR = H // 2          # 6 head pairs
KC = C // 128           # 6 contraction chunks
W = 512                 # T window for scores
NW = T // W             # 2 windows
TCH = T // 128          # 8 T chunks
EXP = mybir.ActivationFunctionType.Exp
IDENT = mybir.ActivationFunctionType.Identity
SCALE = 1.0 / np.sqrt(D)


def _build():
    nc = bacc.Bacc("TRN2", target_bir_lowering=False, debug=False,
                   num_devices=N_CORES)
    xT_d = nc.declare_dram_parameter("xT", [C, T], F16, isOutput=False)
    wqk_d = nc.declare_dram_parameter("w_qk", [C, 2 * C], F16, isOutput=False)
    wv_d = nc.declare_dram_parameter("w_v", [C, C], F16, isOutput=False)
    wp_d = nc.declare_dram_parameter("w_p", [C, C], F32, isOutput=False)
    bqk_d = nc.declare_dram_parameter("b_qk", [128, 12], F32, isOutput=False)
    bv_d = nc.declare_dram_parameter("bv_bc", [128, C], F32, isOutput=False)
    bp_d = nc.declare_dram_parameter("bp_bc", [128, C], F32, isOutput=False)
    kTc_d = nc.declare_dram_parameter("kTc", [C, PFX], F16, isOutput=False)
    vc_d = nc.declare_dram_parameter("vc_aug", [PFX, H, 128], F16, isOutput=False)
    tri_d = nc.declare_dram_parameter("tri", [128, 128], F16, isOutput=False)
    ones_d = nc.declare_dram_parameter("ones2", [66, 64], F32, isOutput=False)
    zer_d = nc.declare_dram_parameter("zeros", [128, T + PFX], F16, isOutput=False)
    out_d = nc.declare_dram_parameter("out", [T, C], F32, isOutput=True)

    with tile.TileContext(nc) as tc, ExitStack() as ctx:
        pers = ctx.enter_context(tc.tile_pool(name="pers", bufs=1))
        wqkp = ctx.enter_context(tc.tile_pool(name="wqkp", bufs=12))
        qkp = ctx.enter_context(tc.tile_pool(name="qkp", bufs=2))
        ep = ctx.enter_context(tc.tile_pool(name="ep", bufs=26))
        khp = ctx.enter_context(tc.tile_pool(name="khp", bufs=4))
        epp = ctx.enter_context(tc.tile_pool(name="epp", bufs=6))
        sbp = ctx.enter_context(tc.tile_pool(name="sbp", bufs=4))
        drp = ctx.enter_context(tc.tile_pool(name="drp", bufs=4))
        dram = ctx.enter_context(tc.tile_pool(name="dram", bufs=4, space="DRAM"))
        rwp = ctx.enter_context(tc.tile_pool(name="rwp", bufs=4))
        op = ctx.enter_context(tc.tile_pool(name="op", bufs=2))
        ps = ctx.enter_context(tc.tile_pool(name="ps", bufs=3, space="PSUM"))
        psp = ctx.enter_context(tc.tile_pool(name="psp", bufs=1, space="PSUM"))
        pyp = ctx.enter_context(tc.tile_pool(name="pyp", bufs=3, space="PSUM"))
        pbp = ctx.enter_context(tc.tile_pool(name="pbp", bufs=1, space="PSUM"))

        # ---- persistent loads (x and w_v first so v-projection starts ASAP) --
        xtb = []
        for k in range(KC):
            t_ = pers.tile([128, T], F16, tag=f"xtb{k}")
            nc.sync.dma_start(t_[:, 0:512], xT_d[128 * k:128 * k + 128, 0:512])
            nc.sync.dma_start(t_[:, 512:T], xT_d[128 * k:128 * k + 128, 512:T])
            xtb.append(t_)
        wv = []
        for k in range(KC):
            t_ = pers.tile([128, C], F16, tag=f"wv{k}")
            nc.sync.dma_start(t_[:], wv_d[128 * k:128 * k + 128, :])
            wv.append(t_)
        bqk = pers.tile([128, 12], F32, tag="bqk")
        nc.sync.dma_start(bqk[:], bqk_d[:])
        bv = pers.tile([128, C], F32, tag="bv")
        nc.sync.dma_start(bv[:], bv_d[:])
        tri = pers.tile([128, 128], F16, tag="tri")
        nc.sync.dma_start(tri[:], tri_d[:])
        vc = pers.tile([PFX, H, 128], F16, tag="vc")
        nc.sync.dma_start(vc[:], vc_d[:])
        ones2 = pers.tile([66, 64], F32R, tag="ones2")
        nc.sync.dma_start(ones2[:], ones_d[:].bitcast(F32R))
        wp = []
        for k in range(KC):
            t_ = pers.tile([128, C], F32R, tag=f"wp{k}")
            nc.sync.dma_start(t_[:], wp_d[128 * k:128 * k + 128, :].bitcast(F32R))
            wp.append(t_)
        bp = pers.tile([128, C], F32, tag="bp")
        nc.sync.dma_start(bp[:], bp_d[:])

        yT = [pers.tile([128, T], F32R, tag=f"yT{p}", name=f"yT{p}")
              for p in range(NPAIR)]

        # ---- v projection: [128, 12, 65] fp16, 65th col per head = 1.0 ----
        vt = []
        for mt in range(TCH):
            v_ = pers.tile([128, H, 128], F16, tag=f"v{mt}")
            nc.vector.memset(v_[:, :, 64:65], 1.0)
            nc.vector.memset(v_[:, :, 65:128], 0.0)
            for n0, nsz in ((0, 512), (512, 256)):
                pv = ps.tile([128, 512], F32, tag="ps", name=f"pv{mt}_{n0}")
                for k in range(KC):
                    nc.tensor.matmul(pv[:, :nsz],
                                     xtb[k][:, 128 * mt:128 * mt + 128],
                                     wv[k][:, n0:n0 + nsz],
                                     start=(k == 0), stop=(k == KC - 1))
                h0, hn = n0 // 64, nsz // 64
                nc.vector.tensor_add(
                    v_[:, h0:h0 + hn, 0:64],
                    pv[:, :nsz].rearrange("a (h d) -> a h d", d=64),
                    bv[:, n0:n0 + nsz].rearrange("a (h d) -> a h d", d=64))
            vt.append(v_)

        # ---- phases ----
        qk_tiles = {}
        ets = {}
        etps = {}
        pys = {}

        def qkproj(p):
            wq = []
            for k in range(KC):
                t_ = wqkp.tile([128, 2, 128], F16, tag="wqk", name=f"wq{p}_{k}")
                src = wqk_d[128 * k:128 * k + 128, :].rearrange(
                    "a (s b) -> a s b", s=2)[:, :, 128 * p:128 * p + 128]
                nc.sync.dma_start(t_[:], src)
                wq.append(t_)
            qT = qkp.tile([128, T], F16, tag="qT", name=f"qT{p}")
            kh = [khp.tile([128, T + PFX], F16, tag="kh", name=f"kh{p}_{s}")
                  for s in range(2)]
            qk_tiles[p] = (qT, kh)
            for s in range(2):
                # head s's k-features live at rows 64s..64s+63 (aligned with
                # its q rows in the pair tile); other 64 rows are zero.
                nc.vector.memset(kh[s][64 - 64 * s:128 - 64 * s, :], 0.0)
                nc.sync.dma_start(
                    kh[s][64 * s:64 * s + 64, T:T + PFX],
                    kTc_d[128 * p + 64 * s:128 * p + 64 * s + 64, :])
            for half in range(2):
                for w in range(NW):
                    pq = ps.tile([128, 512], F32, tag="ps",
                                 name=f"pq{p}_{half}_{w}")
                    for k in range(KC):
                        nc.tensor.matmul(pq[:], wq[k][:, half, :],
                                         xtb[k][:, W * w:W * w + W],
                                         start=(k == 0), stop=(k == KC - 1))
                    if half == 0:
                        nc.vector.tensor_scalar_add(
                            qT[:, W * w:W * w + W], pq[:],
                            bqk[:, p:p + 1])
                    else:
                        for s in range(2):
                            nc.vector.tensor_scalar_add(
                                kh[s][64 * s:64 * s + 64, W * w:W * w + W],
                                pq[64 * s:64 * s + 64, :],
                                bqk[64 * s:64 * s + 64, 6 + p:7 + p])

        def scores(p, s):
            qT, kh = qk_tiles[p]
            qs = qT
            ks = kh[s]
            for c in range(NW):
                for r in range(4 * c + 4):
                    e_ = ep.tile([128, W], F16, tag="et", name=f"et{p}_{s}_{c}_{r}")
                    pss = ps.tile([128, 512], F32, tag="ps",
                                  name=f"pss{p}_{s}_{c}_{r}")
                    if r >= 4 * c:  # diagonal band tile
                        j0 = 128 * r - W * c
                        nc.tensor.matmul(pss[:, j0:W], ks[:, 128 * r:128 * r + 128],
                                         qs[:, W * c + j0:W * (c + 1)],
                                         start=True, stop=True)
                        nc.scalar.activation(e_[:, j0:W], pss[:, j0:W], EXP,
                                             scale=float(SCALE))
                        nc.vector.tensor_mul(e_[:, j0:j0 + 128],
                                             e_[:, j0:j0 + 128], tri[:])
                    else:
                        nc.tensor.matmul(pss[:], ks[:, 128 * r:128 * r + 128],
                                         qs[:, W * c:W * (c + 1)],
                                         start=True, stop=True)
                        nc.scalar.activation(e_[:], pss[:], EXP,
                                             scale=float(SCALE))
                    ets[(p, s, c, r)] = e_
                pp = psp.tile([PFX, 512], F32, tag="psp", name=f"pp{p}_{s}_{c}")
                nc.tensor.matmul(pp[:], ks[:, T:T + PFX],
                                 qs[:, W * c:W * (c + 1)], start=True, stop=True)
                ep_ = epp.tile([PFX, W], F16, tag="etp", name=f"etp{p}_{s}_{c}")
                nc.scalar.activation(ep_[:], pp[:], EXP, scale=float(SCALE))
                etps[(p, s, c)] = ep_

        def av(p, s, c):
            """y^T accumulation: py[0:65, t] = [y(64 dims); denom] for head."""
            h = 2 * p + s
            py = pyp.tile([128, W], F32, tag="py", name=f"py{p}_{s}_{c}")
            pys[(p, s, c)] = py
            dst = py[:, :]
            for r in range(4 * c + 4):
                tstart = max(0, 128 * r - W * c)
                nc.tensor.matmul(dst[:, tstart:W],
                                 vt[r][:, h, :],
                                 ets[(p, s, c, r)][:, tstart:W],
                                 start=(r == 0), stop=False)
            nc.tensor.matmul(dst[:, :], vc[:, h, :],
                             etps[(p, s, c)][:, :], start=False, stop=True)

        def norm(p, c):
            """Normalize both heads of the pair for window c into yT[p].

            The denom row [1, W] is reshaped to [128, 4] via DMA so the
            reciprocal runs partition-parallel, DMA'd back as an f32r row,
            broadcast to [64, W] via a K=2 f32r matmul, and multiplied in
            with a shifted output base for head 1."""
            for s in range(2):
                py = pys[(p, s, c)]
                sb = sbp.tile([66, W], F32, tag="sb", name=f"sb{p}_{s}_{c}")
                nc.vector.tensor_copy(sb[:], py[0:66, :])
                dsc = dram.tile([2, W], F32, tag="dsc", name=f"dsc{p}_{s}_{c}")
                nc.sync.dma_start(dsc[0:1, :], sb[64:65, :])
                drow = drp.tile([128, 8], F32, tag="dr", name=f"dr{p}_{s}_{c}")
                nc.vector.memset(drow[:, 4:8], 1.0)
                nc.sync.dma_start(
                    drow[:, 0:4],
                    dsc[0:1, :].rearrange("a (p f) -> (a p) f", p=128))
                rrec = drp.tile([128, 8], F32, tag="rr", name=f"rr{p}_{s}_{c}")
                nc.vector.reciprocal(rrec[:], drow[:])
                dsc2 = dram.tile([2, W], F32, tag="dsc2", name=f"ds2{p}_{s}_{c}")
                nc.sync.dma_start(
                    dsc2[:].rearrange("r (p f) -> p r f", p=128),
                    rrec[:].rearrange("p (r f) -> p r f", r=2))
                rrow = rwp.tile([66, W], F32R, tag="rw", name=f"rw{p}_{s}_{c}")
                nc.sync.dma_start(rrow[64:66, :], dsc2[:].bitcast(F32R))
                pb = pbp.tile([64, W], F32, tag="pb", name=f"pd{p}_{s}_{c}")
                nc.tensor.matmul(pb[:], ones2[64:66, :], rrow[64:66, :],
                                 start=True, stop=True)
                nc.vector.tensor_mul(yT[p][64 * s:64 * s + 64, W * c:W * c + W],
                                     sb[0:64, :], pb[:])

        def outproj(mts):
            for mt in mts:
                osb = op.tile([128, C], F32, tag="osb", name=f"osb{mt}")
                for n0, nsz in ((0, 512), (512, 256)):
                    po = ps.tile([128, 512], F32, tag="ps", name=f"po{mt}_{n0}")
                    for kp in range(NPAIR):
                        nc.tensor.matmul(po[:, :nsz],
                                         yT[kp][:, 128 * mt:128 * mt + 128],
                                         wp[kp][:, n0:n0 + nsz],
                                         start=(kp == 0), stop=(kp == NPAIR - 1))
                    nc.vector.tensor_add(osb[:, n0:n0 + nsz], po[:, :nsz],
                                         bp[:, n0:n0 + nsz])
                nc.sync.dma_start(out_d[128 * mt:128 * mt + 128, :], osb[:])

        # ---- emission schedule ----
        qkproj(0)
        scores(0, 0)
        for p in range(NPAIR):
            scores(p, 1)
            if p < NPAIR - 1:
                qkproj(p + 1)
            for c in range(NW):
                av(p, 0, c)
                av(p, 1, c)
                norm(p, c)
            if p < NPAIR - 1:
                scores(p + 1, 0)
        outproj(range(TCH))

    nc.finalize()
    return nc


def _ones2():
    o = np.zeros((66, 64), np.float32)
    o[64, :] = 1.0
    return o


def _prep_inputs(x, kv_cvec, w_attn, b_attn, w_proj, b_proj):
    x = np.asarray(x, np.float32)
    kv_cvec = np.asarray(kv_cvec, np.float32)
    w_attn = np.asarray(w_attn, np.float32)
    b_attn = np.asarray(b_attn, np.float32)
    w_proj = np.asarray(w_proj, np.float32)
    b_proj = np.asarray(b_proj, np.float32)

    shared = {
        "w_qk": np.ascontiguousarray(w_attn[:, :2 * C]).astype(np.float16),
        "w_v": np.ascontiguousarray(w_attn[:, 2 * C:]).astype(np.float16),
        "w_p": np.ascontiguousarray(w_proj),
        "b_qk": np.ascontiguousarray(b_attn[:2 * C].reshape(12, 128).T),
        "bv_bc": np.ascontiguousarray(
            np.broadcast_to(b_attn[2 * C:], (128, C))),
        "bp_bc": np.ascontiguousarray(np.broadcast_to(b_proj, (128, C))),
        "tri": (np.arange(128)[:, None] <= np.arange(128)[None, :]
                ).astype(np.float16),
        "ones2": _ones2(),
        "zeros": np.zeros((128, T + PFX), np.float16),
    }
    in_maps = []
    for b in range(N_CORES):
        vc_aug = np.zeros((PFX, H, 128), np.float32)
        vc_aug[:, :, :64] = kv_cvec[b][:, C:].reshape(PFX, H, D)
        vc_aug[:, :, 64] = 1.0
        m = dict(shared)
        m["xT"] = np.ascontiguousarray(x[b].T).astype(np.float16)
        m["kTc"] = np.ascontiguousarray(kv_cvec[b][:, :C].T
                                        ).astype(np.float16)
        m["vc_aug"] = vc_aug.astype(np.float16)
        in_maps.append(m)
    return in_maps


_NC_CACHE = {}


def run_hw(trace=False, **inputs):
    """Build+compile+run on 8 NeuronCores; returns (out [8,1024,768], results)."""
    if "nc" not in _NC_CACHE:
        _NC_CACHE["nc"] = _build()
    nc = _NC_CACHE["nc"]
    in_maps = _prep_inputs(**inputs)
    res = run_bass_kernel_spmd(nc, in_maps, list(range(N_CORES)), trace=trace)
    out = np.stack([res.results[b]["out"] for b in range(N_CORES)])
    return out, res


def kernel(**inputs):
    out, _ = run_hw(trace=False, **inputs)
    return out
